# revision 31
# baseline (speedup 1.0000x reference)
"""Trainium2 Bass kernel: bidirectional RAFT-style correlation pyramid lookup
(AMT BidirCorrBlock + _corr_scale_lookup; B=1, D=128, H=60, W=108, r=3, L=4).

Design (8 NeuronCores, SPMD, no collectives):
  - Host computes lookup coords from flow/embt and 2D-buckets the queries:
    sort by y into 8 bands (one per core), then by x into 7 buckets of ~116
    queries.  Each bucket's correlation support is a small 2D tile per level
    (~600-820 cols total vs ~2700 for 1D y-slabs).
  - Host packs, per core, the needed image-tile columns of the (pooled)
    feature pyramids into flat "imgslab" tensors, so the device matmuls are
    plain contiguous [128q x QS] slabs (query features pre-scaled 1/sqrt(D)).
  - Per (chunk, dir): 2 matmuls -> one 2-bank PSUM tile -> ONE f32->bf16
    drain -> ONE bf16 DMA write to a per-(chunk,dir) DRAM tensor
    (query-major rows).
  - Per (chunk, dir, level): ONE indirect DMA band gather with a single
    offset per partition (the only form the real DGE supports): 7*tw+8
    contiguous elems cover a query's full 8x8 window at row stride tw.
    Because tile bounds are UNCLAMPED (OOB rows/cols are host-packed
    zeros), every band lies inside its own query's level block -- no DRAM
    zeroing or offset clamping needed.
  - Blend: 8 cheap DVE patch-extract copies + 6 DVE ops/chunk (separable
    bilinear); the OOB masks are folded into host-computed per-tap weights
    (w0x/w1x per (g,b'), w0y/w1y materialized per (g,a',b') so every op
    runs in 2x 16-bit mode).  The last chunk blends per-dir halves to
    shorten the tail.
  - Host reassembles [1, 396, 60, 108] (channel reorder + per-bucket query
    permutation + flow passthrough).  Program is compiled per bucket-shape
    key (cached across calls).
"""

import sys

import numpy as np

sys.path.insert(0, "/opt/trn_rl_repo")

import concourse.bass as bass
import concourse.bacc as bacc
import concourse.mybir as mybir
from concourse.bass import IndirectOffsetOnAxis
from concourse.tile import TileContext

F32 = mybir.dt.float32
BF16 = mybir.dt.bfloat16
I32 = mybir.dt.int32
OP = mybir.AluOpType
ACT_COPY = mybir.ActivationFunctionType.Copy

# problem geometry
H, W = 60, 108
N = H * W            # 6480
D = 128
NCORES = 8
NCH = 7              # x-buckets per core per dir
CH = 128             # query slots per chunk (partition dim)
NQP = NCH * CH       # 896
NPB = N // NCORES    # 810 queries per core per dir
NL = 4
HL = [60, 30, 15, 7]
WL = [108, 54, 27, 13]

INV_SQRT_D = np.float32(1.0) / np.sqrt(np.float32(D))


def plan_cfg(flow0, flow1, embt):
    """Host plan: 2D buckets, per-(chunk,dir,level) padded tile shapes."""
    ev = np.float32(np.asarray(embt).reshape(-1)[0])
    tsc = [np.float32(1.0) / ev, np.float32(1.0) / (np.float32(1.0) - ev)]
    wq = (np.arange(N) % W).astype(np.float32)
    hq = (np.arange(N) // W).astype(np.float32)

    xs, ys = [], []                     # [d] -> f32 coords per query
    buckets = [[[None] * NCH for _ in range(NCORES)] for _ in range(2)]
    r0 = np.zeros((NCORES, NCH, 2, NL), np.int64)
    c0 = np.zeros((NCORES, NCH, 2, NL), np.int64)
    rows = np.zeros((NCORES, NCH, 2, NL), np.int64)
    tws = np.zeros((NCORES, NCH, 2, NL), np.int64)
    for d in range(2):
        fl = (flow1 if d == 0 else flow0).reshape(2, N).astype(np.float32)
        x = (wq + fl[0] * tsc[d]).astype(np.float32)
        y = (hq + fl[1] * tsc[d]).astype(np.float32)
        xs.append(x)
        ys.append(y)
        order = np.argsort(y, kind="stable")
        for m in range(NCORES):
            band = order[m * NPB:(m + 1) * NPB]
            bx = band[np.argsort(x[band], kind="stable")]
            for c in range(NCH):
                qs = bx[NPB * c // NCH:NPB * (c + 1) // NCH]
                buckets[d][m][c] = qs
                for l in range(NL):
                    sc = np.float32(0.5 ** l)
                    fx = np.floor(x[qs] * sc)
                    fy = np.floor(y[qs] * sc)
                    # UNCLAMPED bounds: the tile always contains the full
                    # 8x8 window of every query (OOB rows/cols are host-
                    # packed zeros), so band reads never leave their block
                    a = int(fy.min()) - 3
                    b = int(fy.max()) + 4
                    e = int(fx.min()) - 3
                    f = int(fx.max()) + 4
                    r0[m, c, d, l] = a
                    c0[m, c, d, l] = e
                    rows[m, c, d, l] = b - a + 1
                    tws[m, c, d, l] = f - e + 1

    rows_p = rows.max(axis=0)           # [NCH, 2, NL] program shapes
    tws_p = tws.max(axis=0)

    loff = np.zeros((NCH, 2, NL + 1), np.int64)
    QS = np.zeros((NCH, 2), np.int64)
    for c in range(NCH):
        for d in range(2):
            off = 0
            for l in range(NL):
                loff[c, d, l] = off
                off += rows_p[c, d, l] * tws_p[c, d, l]
            loff[c, d, NL] = off
            QS[c, d] = (off + 31) // 32 * 32      # 4096-elem block alignment

    ext = CH * QS                                 # per-(c,d) DRAM tensor size
    # band geometry: one contiguous run of 7*tw+8 covers the 8x8 window
    BW = 7 * tws_p + 8                            # [NCH, 2, NL]
    BOFF = np.zeros((NCH, 2, NL), np.int64)       # band offsets in band tile
    BANDW = np.zeros(NCH, np.int64)
    for c in range(NCH):
        off = 0
        for d in range(2):
            for l in range(NL):
                BOFF[c, d, l] = off
                off += int(BW[c, d, l])
        BANDW[c] = off + 32          # slack for the 8*tw extraction views

    soff = np.zeros((NCH, 2), np.int64)           # imgslab offsets
    ST = [0, 0]
    for d in range(2):
        off = 0
        for c in range(NCH):
            soff[c, d] = off
            off += QS[c, d]
        ST[d] = off

    cfg = {
        "ev": ev, "tsc": tsc, "xs": xs, "ys": ys, "buckets": buckets,
        "r0": r0, "c0": c0, "rows_p": rows_p, "tws_p": tws_p,
        "loff": loff, "QS": QS, "ext": ext, "BW": BW, "BOFF": BOFF,
        "BANDW": BANDW, "soff": soff, "ST": ST,
    }
    cfg["key"] = (tuple(rows_p.ravel()), tuple(tws_p.ravel()),
                  tuple(QS.ravel()))
    return cfg


# static engine rotation tables (index by serial counters)
# drains: gpsimd cannot access PSUM -> DVE/Act only; Pool is reserved for
# the 56 band gathers (the bottleneck queue)
DRAIN_ENG = ["A", "A", "A", "V", "A", "A", "V", "A", "A", "V", "A", "A",
             "V", "A"]
WRITE_ENG = ["S", "A", "S", "A", "S", "A", "S", "A", "S", "A", "S", "A",
             "S", "A"]
IMG_ENG = ["S", "A", "S", "A", "S", "A", "S", "A", "S", "A", "S", "A",
           "S", "A"]


def build_nc(cfg):
    nc = bacc.Bacc()
    QS, ext, soff, ST = cfg["QS"], cfg["ext"], cfg["soff"], cfg["ST"]
    BW, BOFF, BANDW = cfg["BW"], cfg["BOFF"], cfg["BANDW"]
    tws_p = cfg["tws_p"]
    QSMAX = int(QS.max())
    BANDMAX = int(BANDW.max())

    f0qp = nc.declare_dram_parameter("f0q", [D, NQP], BF16, isOutput=False)
    f1qp = nc.declare_dram_parameter("f1q", [D, NQP], BF16, isOutput=False)
    img0p = nc.declare_dram_parameter("imgs0", [D, int(ST[0])], BF16,
                                      isOutput=False)
    img1p = nc.declare_dram_parameter("imgs1", [D, int(ST[1])], BF16,
                                      isOutput=False)
    offp = nc.declare_dram_parameter("offs", [128, NCH * 8], I32,
                                     isOutput=False)
    w0xp = nc.declare_dram_parameter("w0x", [128, NCH * 56], BF16,
                                     isOutput=False)
    w1xp = nc.declare_dram_parameter("w1x", [128, NCH * 56], BF16,
                                     isOutput=False)
    w0yp = nc.declare_dram_parameter("w0y", [128, NCH * 392], BF16,
                                     isOutput=False)
    w1yp = nc.declare_dram_parameter("w1y", [128, NCH * 392], BF16,
                                     isOutput=False)
    outp = nc.declare_dram_parameter("out", [NQP, 392], BF16, isOutput=True)

    with TileContext(nc) as tc:
        cpool = tc.alloc_tile_pool(name="cpool", bufs=1)
        dpool = tc.alloc_tile_pool(name="dpool", bufs=1, space="DRAM")
        ppool = tc.alloc_tile_pool(name="ppool", bufs=4, space="PSUM")
        spool = tc.alloc_tile_pool(name="spool", bufs=3)
        gpool = tc.alloc_tile_pool(name="gpool", bufs=3)
        bpool = tc.alloc_tile_pool(name="bpool", bufs=2)

        ENG = {"V": nc.vector, "A": nc.scalar, "P": nc.gpsimd, "S": nc.sync}

        # ---- input loads ----
        f0qs = cpool.tile([D, NQP], BF16, name="f0qs")
        f1qs = cpool.tile([D, NQP], BF16, name="f1qs")
        img0s = cpool.tile([D, int(ST[0])], BF16, name="img0s")
        img1s = cpool.tile([D, int(ST[1])], BF16, name="img1s")
        offs = cpool.tile([128, NCH * 8], I32, name="offs")
        w0xs = cpool.tile([128, NCH * 56], BF16, name="w0xs")
        w1xs = cpool.tile([128, NCH * 56], BF16, name="w1xs")
        w0ys = cpool.tile([128, NCH * 392], BF16, name="w0ys")
        w1ys = cpool.tile([128, NCH * 392], BF16, name="w1ys")

        def load_img(c, d):
            imgs, imgp = ((img0s, img0p), (img1s, img1p))[d]
            s0 = int(soff[c, d])
            q = int(QS[c, d])
            eng = nc.sync if d == 0 else nc.scalar
            eng.dma_start(out=imgs[:, s0:s0 + q], in_=imgp[:, s0:s0 + q])

        # head: only what chunk 0/1 need, so the first write lands early
        nc.scalar.dma_start(out=offs[:], in_=offp[:])
        load_img(0, 0)
        nc.sync.dma_start(out=f0qs[:], in_=f0qp[:])
        nc.scalar.dma_start(out=f1qs[:], in_=f1qp[:])
        load_img(0, 1)
        load_img(1, 0)
        load_img(1, 1)
        # warm the Act activation table after the head loads (it occupies
        # the Act queue for 1283ns; the first Act drain is at ~4us)
        zt = cpool.tile([128, 8], BF16, name="zt")
        zt2 = cpool.tile([128, 8], BF16, name="zt2")
        nc.vector.memset(zt[:], 0.0)
        nc.scalar.activation(out=zt2[:], in_=zt[:], func=ACT_COPY)

        # ---- per-(chunk,dir) DRAM tensors (offset-0 for indirect DMA) ----
        drams = {(c, d): dpool.tile([int(ext[c, d]), 1], BF16,
                                    name=f"pyr{c}_{d}")
                 for c in range(NCH) for d in range(2)}

        bands = {}

        def emit_compute(c):
            bandt = bpool.tile([128, BANDMAX], BF16, name="band", tag="band",
                               bufs=3)
            bands[c] = bandt
            if c + 2 < NCH:
                load_img(c + 2, 0)
                load_img(c + 2, 1)
            for d in range(2):
                q = int(QS[c, d])
                s0 = int(soff[c, d])
                lhsT = (f0qs if d == 0 else f1qs)[:, c * CH:(c + 1) * CH]
                imgs = img0s if d == 0 else img1s
                ps = ppool.tile([128, 1024], F32, name="ps", tag="ps")
                for p0 in range(0, q, 512):
                    plen = min(512, q - p0)
                    nc.tensor.matmul(
                        out=ps[:, p0:p0 + plen], lhsT=lhsT,
                        rhs=imgs[:, s0 + p0:s0 + p0 + plen],
                        start=True, stop=True)
                slab = spool.tile([128, QSMAX], BF16, name="slab", tag="slab")
                dr = drams[(c, d)]
                # drain d0 on DVE, d1 on Act; write d0 on SP, d1 on Act
                if d == 0:
                    nc.vector.tensor_copy(out=slab[:, 0:q], in_=ps[:, 0:q])
                    weng = nc.sync
                else:
                    nc.scalar.activation(out=slab[:, 0:q], in_=ps[:, 0:q],
                                         func=ACT_COPY)
                    weng = nc.scalar
                dst = bass.AP(dr[:, 0].tensor, dr[:, 0].offset,
                              [[q, CH], [1, q]])
                weng.dma_start(out=dst, in_=slab[:, 0:q])
                # per-level band gathers: ONE offset per partition (the only
                # indirect-DMA form the real DGE supports)
                inv = bass.AP(dr[:, 0].tensor, dr[:, 0].offset,
                              [[4096, int(ext[c, d]) // 4096], [1, 4096]])
                for l in range(NL):
                    g = d * 4 + l
                    bo = int(BOFF[c, d, l])
                    nc.gpsimd.indirect_dma_start(
                        out=bandt[:, bo:bo + int(BW[c, d, l])],
                        out_offset=None, in_=inv,
                        in_offset=IndirectOffsetOnAxis(
                            ap=offs[:, c * 8 + g:c * 8 + g + 1], axis=1))
            if c == 1:
                # weight tables: needed first by blend(0) (~9.5k); emitting
                # them here keeps the scheduler from hoisting the big w0y/
                # w1y transfers into chunk 0's critical write bubble
                nc.sync.dma_start(out=w0xs[:], in_=w0xp[:])
                nc.scalar.dma_start(out=w1xs[:], in_=w1xp[:])
                nc.sync.dma_start(out=w0ys[:], in_=w0yp[:])
                nc.scalar.dma_start(out=w1ys[:], in_=w1yp[:])

        def emit_blend(c, split=False):
            bandt = bands.pop(c)
            pt = gpool.tile([128, 512], BF16, name="pt", tag="pt")
            pbv = pt[:].rearrange("p (g a b) -> p g a b", g=8, a=8)
            w0xv = w0xs[:, c * 56:(c + 1) * 56].rearrange(
                "p (g b) -> p g b", g=8).unsqueeze(2).broadcast_to(
                [128, 8, 8, 7])
            w1xv = w1xs[:, c * 56:(c + 1) * 56].rearrange(
                "p (g b) -> p g b", g=8).unsqueeze(2).broadcast_to(
                [128, 8, 8, 7])
            w0yv = w0ys[:, c * 392:(c + 1) * 392].rearrange(
                "p (g a b) -> p g a b", g=8, a=7)
            w1yv = w1ys[:, c * 392:(c + 1) * 392].rearrange(
                "p (g a b) -> p g a b", g=8, a=7)
            t0 = bpool.tile([128, 448], BF16, name="t0", tag="t0")
            t1 = bpool.tile([128, 448], BF16, name="t1", tag="t1")
            px = bpool.tile([128, 448], BF16, name="px", tag="px")
            t0v = t0[:].rearrange("p (g a b) -> p g a b", g=8, a=8)
            t1v = t1[:].rearrange("p (g a b) -> p g a b", g=8, a=8)
            pxv = px[:].rearrange("p (g a b) -> p g a b", g=8, a=8)
            u0 = bpool.tile([128, 392], BF16, name="u0", tag="u0")
            u1 = bpool.tile([128, 392], BF16, name="u1", tag="u1")
            ot = bpool.tile([128, 392], BF16, name="ot", tag="ot")
            u0v = u0[:].rearrange("p (g a b) -> p g a b", g=8, a=7)
            u1v = u1[:].rearrange("p (g a b) -> p g a b", g=8, a=7)
            otv = ot[:].rearrange("p (g a b) -> p g a b", g=8, a=7)

            def half(d):
                # extract this dir's 4 patches, then blend its g-slice
                for l in range(NL):
                    g = d * 4 + l
                    tw = int(tws_p[c, d, l])
                    bo = int(BOFF[c, d, l])
                    bv = bandt[:, bo:bo + 8 * tw].rearrange(
                        "p (a w) -> p a w", a=8)[:, :, 0:8]
                    nc.vector.tensor_copy(out=pbv[:, g], in_=bv)
                gs = slice(d * 4, d * 4 + 4)
                nc.vector.tensor_tensor(out=t0v[:, gs], in0=pbv[:, gs, :, 0:7],
                                        in1=w0xv[:, gs], op=OP.mult)
                nc.vector.tensor_tensor(out=t1v[:, gs], in0=pbv[:, gs, :, 1:8],
                                        in1=w1xv[:, gs], op=OP.mult)
                nc.vector.tensor_tensor(out=pxv[:, gs], in0=t0v[:, gs],
                                        in1=t1v[:, gs], op=OP.add)
                nc.vector.tensor_tensor(out=u0v[:, gs],
                                        in0=pxv[:, gs, 0:7, :],
                                        in1=w0yv[:, gs], op=OP.mult)
                nc.vector.tensor_tensor(out=u1v[:, gs],
                                        in0=pxv[:, gs, 1:8, :],
                                        in1=w1yv[:, gs], op=OP.mult)
                nc.vector.tensor_tensor(out=otv[:, gs], in0=u0v[:, gs],
                                        in1=u1v[:, gs], op=OP.add)
                nc.sync.dma_start(
                    out=outp[c * CH:(c + 1) * CH, d * 196:(d + 1) * 196],
                    in_=ot[:, d * 196:(d + 1) * 196])

            if split:
                half(0)
                half(1)
                return
            for d in range(2):
                for l in range(NL):
                    g = d * 4 + l
                    tw = int(tws_p[c, d, l])
                    bo = int(BOFF[c, d, l])
                    bv = bandt[:, bo:bo + 8 * tw].rearrange(
                        "p (a w) -> p a w", a=8)[:, :, 0:8]
                    nc.vector.tensor_copy(out=pbv[:, g], in_=bv)
            nc.vector.tensor_tensor(out=t0v, in0=pbv[:, :, :, 0:7],
                                    in1=w0xv, op=OP.mult)
            nc.vector.tensor_tensor(out=t1v, in0=pbv[:, :, :, 1:8],
                                    in1=w1xv, op=OP.mult)
            nc.vector.tensor_tensor(out=pxv, in0=t0v, in1=t1v, op=OP.add)
            nc.vector.tensor_tensor(out=u0v, in0=pxv[:, :, 0:7, :],
                                    in1=w0yv, op=OP.mult)
            nc.vector.tensor_tensor(out=u1v, in0=pxv[:, :, 1:8, :],
                                    in1=w1yv, op=OP.mult)
            nc.vector.tensor_tensor(out=otv, in0=u0v, in1=u1v, op=OP.add)
            nc.sync.dma_start(out=outp[c * CH:(c + 1) * CH, :], in_=ot[:])

        for c in range(NCH):
            emit_compute(c)
            if c >= 1:
                emit_blend(c - 1)
        emit_blend(NCH - 1, split=True)

        for pool in (bpool, gpool, spool, ppool, dpool, cpool):
            pool.release()

    nc.finalize()
    return nc


def host_tables(cfg, m):
    """Per-core gather offsets and folded bilinear/mask weights."""
    import ml_dtypes
    bf = ml_dtypes.bfloat16
    offs = np.zeros((128, NCH * 8), np.int32)
    w0x = np.zeros((128, NCH * 56), np.float32)
    w1x = np.zeros((128, NCH * 56), np.float32)
    w0y = np.zeros((128, NCH * 392), np.float32)
    w1y = np.zeros((128, NCH * 392), np.float32)
    QS, ext, loff, BW = cfg["QS"], cfg["ext"], cfg["loff"], cfg["BW"]
    av = np.arange(8)
    for c in range(NCH):
        for d in range(2):
            qs = cfg["buckets"][d][m][c]
            n = len(qs)
            rows_i = np.arange(n)
            q = int(QS[c, d])
            for l in range(NL):
                g = d * 4 + l
                sc = np.float32(0.5 ** l)
                xl = (cfg["xs"][d][qs] * sc).astype(np.float32)
                yl = (cfg["ys"][d][qs] * sc).astype(np.float32)
                fx = np.floor(xl)
                fy = np.floor(yl)
                wx = (xl - fx).astype(np.float32)
                wy = (yl - fy).astype(np.float32)
                stx = (fx - 3).astype(np.int64)
                sty = (fy - 3).astype(np.int64)
                r0 = int(cfg["r0"][m, c, d, l])
                cc0 = int(cfg["c0"][m, c, d, l])
                tw = int(cfg["tws_p"][c, d, l])
                # ONE band offset per query: 7*tw+8 contiguous elems cover
                # the 8 window rows at stride tw.  With unclamped tile
                # bounds the band is always inside this query's level
                # block; the clip below is a pure safety net.
                off = (rows_i * q + int(loff[c, d, l])
                       + (sty - r0) * tw + (stx - cc0))
                off = np.clip(off, 0, int(ext[c, d]) - int(BW[c, d, l]))
                offs[:n, c * 8 + g] = off
                # validity masks
                rv = ((sty[:, None] + av[None, :] >= 0)
                      & (sty[:, None] + av[None, :] <= HL[l] - 1)).astype(
                    np.float32)                                   # [n, 8]
                cv = ((stx[:, None] + av[None, :] >= 0)
                      & (stx[:, None] + av[None, :] <= WL[l] - 1)).astype(
                    np.float32)                                   # [n, 8]
                # x weights with col mask folded: [n, 7]
                w0x[:n, c * 56 + g * 7:c * 56 + g * 7 + 7] = \
                    (1.0 - wx)[:, None] * cv[:, 0:7]
                w1x[:n, c * 56 + g * 7:c * 56 + g * 7 + 7] = \
                    wx[:, None] * cv[:, 1:8]
                # y weights with row mask folded, materialized over b': [n,7,7]
                o = c * 392 + g * 49
                w0y[:n, o:o + 49] = np.repeat(
                    (1.0 - wy)[:, None] * rv[:, 0:7], 7, axis=1)
                w1y[:n, o:o + 49] = np.repeat(
                    wy[:, None] * rv[:, 1:8], 7, axis=1)
    return (offs, w0x.astype(bf), w1x.astype(bf), w0y.astype(bf),
            w1y.astype(bf))


def host_prepare(cfg, fmap0, fmap1, flow0, flow1, embt):
    import ml_dtypes
    bf = ml_dtypes.bfloat16
    f0 = fmap0.reshape(D, N).astype(np.float32)
    f1 = fmap1.reshape(D, N).astype(np.float32)

    def pyramid(f):
        cur = f.reshape(D, H, W)
        out = [cur]
        for l in range(1, NL):
            h, w = cur.shape[1] // 2, cur.shape[2] // 2
            cur = cur[:, :2 * h, :2 * w].reshape(D, h, 2, w, 2).mean((2, 4))
            out.append(cur)
        return out

    pyr = [pyramid(f1), pyramid(f0)]     # d=0 looks into fmap1, d=1 fmap0

    f0s = (f0 * INV_SQRT_D).astype(np.float32)
    f1s = (f1 * INV_SQRT_D).astype(np.float32)

    QS, soff, ST = cfg["QS"], cfg["soff"], cfg["ST"]
    loff = cfg["loff"]

    in_maps = []
    for m in range(NCORES):
        f0qA = np.zeros((D, NQP), np.float32)
        f1qA = np.zeros((D, NQP), np.float32)
        imgs = [np.zeros((D, int(ST[0])), np.float32),
                np.zeros((D, int(ST[1])), np.float32)]
        for c in range(NCH):
            for d in range(2):
                qs = cfg["buckets"][d][m][c]
                n = len(qs)
                if d == 0:
                    f0qA[:, c * CH:c * CH + n] = f0s[:, qs]
                else:
                    f1qA[:, c * CH:c * CH + n] = f1s[:, qs]
                for l in range(NL):
                    r0 = int(cfg["r0"][m, c, d, l])
                    cc0 = int(cfg["c0"][m, c, d, l])
                    rp = int(cfg["rows_p"][c, d, l])
                    tp = int(cfg["tws_p"][c, d, l])
                    src = pyr[d][l]
                    rs, re = max(0, r0), min(HL[l], r0 + rp)
                    cs, ce = max(0, cc0), min(WL[l], cc0 + tp)
                    blk = np.zeros((D, rp, tp), np.float32)
                    if re > rs and ce > cs:
                        blk[:, rs - r0:re - r0, cs - cc0:ce - cc0] = \
                            src[:, rs:re, cs:ce]
                    o = int(soff[c, d]) + int(loff[c, d, l])
                    imgs[d][:, o:o + rp * tp] = blk.reshape(D, rp * tp)
        offs, w0x, w1x, w0y, w1y = host_tables(cfg, m)
        in_maps.append({
            "f0q": np.ascontiguousarray(f0qA.astype(bf)),
            "f1q": np.ascontiguousarray(f1qA.astype(bf)),
            "imgs0": np.ascontiguousarray(imgs[0].astype(bf)),
            "imgs1": np.ascontiguousarray(imgs[1].astype(bf)),
            "offs": offs, "w0x": w0x, "w1x": w1x, "w0y": w0y, "w1y": w1y,
        })
    return in_maps


def assemble(cfg, results, flow0, flow1):
    corr = np.zeros((392, N), np.float32)
    res = [np.asarray(r["out"]).astype(np.float32) for r in results]
    for d in range(2):
        for m in range(NCORES):
            for c in range(NCH):
                qs = cfg["buckets"][d][m][c]
                n = len(qs)
                t = res[m][c * CH:c * CH + n].reshape(n, 8, 49)
                for l in range(NL):
                    ch0 = d * 196 + l * 49
                    corr[ch0:ch0 + 49, qs] = t[:, d * 4 + l, :].T
    full = np.concatenate(
        [corr.reshape(1, 392, H, W),
         flow0.astype(np.float32), flow1.astype(np.float32)], axis=1)
    return full


_CACHED = {}
LAST_NC = None


def kernel(fmap0, fmap1, flow0, flow1, embt):
    global LAST_NC
    from concourse.bass_utils import run_bass_kernel_spmd

    fmap0 = np.asarray(fmap0)
    fmap1 = np.asarray(fmap1)
    flow0 = np.asarray(flow0)
    flow1 = np.asarray(flow1)
    embt = np.asarray(embt)

    cfg = plan_cfg(flow0, flow1, embt)
    if cfg["key"] not in _CACHED:
        _CACHED[cfg["key"]] = build_nc(cfg)
    nc = _CACHED[cfg["key"]]
    LAST_NC = nc

    in_maps = host_prepare(cfg, fmap0, fmap1, flow0, flow1, embt)
    res = run_bass_kernel_spmd(nc, in_maps, core_ids=list(range(NCORES)))
    return assemble(cfg, res.results, flow0, flow1)


# revision 32
# speedup vs baseline: 1.0569x; 1.0569x over previous
"""Trainium2 Bass kernel: bidirectional RAFT-style correlation pyramid lookup
(AMT BidirCorrBlock + _corr_scale_lookup; B=1, D=128, H=60, W=108, r=3, L=4).

Design (8 NeuronCores, SPMD, no collectives):
  - Host computes lookup coords from flow/embt and 2D-buckets the queries:
    sort by y into 8 bands (one per core), then by x into 7 buckets of ~116
    queries.  Each bucket's correlation support is a small 2D tile per level
    (~600-820 cols total vs ~2700 for 1D y-slabs).
  - Host packs, per core, the needed image-tile columns of the (pooled)
    feature pyramids into flat "imgslab" tensors, so the device matmuls are
    plain contiguous [128q x QS] slabs (query features pre-scaled 1/sqrt(D)).
  - Per (chunk, dir): 2 matmuls -> one 2-bank PSUM tile -> ONE f32->bf16
    drain -> ONE bf16 DMA write to a per-(chunk,dir) DRAM tensor
    (query-major rows).
  - Per (chunk, dir, level): ONE indirect DMA band gather with a single
    offset per partition (the only form the real DGE supports): 7*tw+8
    contiguous elems cover a query's full 8x8 window at row stride tw.
    Because tile bounds are UNCLAMPED (OOB rows/cols are host-packed
    zeros), every band lies inside its own query's level block -- no DRAM
    zeroing or offset clamping needed.
  - Blend: 8 cheap DVE patch-extract copies + 6 DVE ops/chunk (separable
    bilinear); the OOB masks are folded into host-computed per-tap weights
    (w0x/w1x per (g,b'), w0y/w1y materialized per (g,a',b') so every op
    runs in 2x 16-bit mode).  The last chunk blends per-dir halves to
    shorten the tail.
  - Host reassembles [1, 396, 60, 108] (channel reorder + per-bucket query
    permutation + flow passthrough).  Program is compiled per bucket-shape
    key (cached across calls).
"""

import sys

import numpy as np

sys.path.insert(0, "/opt/trn_rl_repo")

import concourse.bass as bass
import concourse.bacc as bacc
import concourse.mybir as mybir
from concourse.bass import IndirectOffsetOnAxis
from concourse.tile import TileContext

F32 = mybir.dt.float32
BF16 = mybir.dt.bfloat16
I32 = mybir.dt.int32
OP = mybir.AluOpType
ACT_COPY = mybir.ActivationFunctionType.Copy

# problem geometry
H, W = 60, 108
N = H * W            # 6480
D = 128
NCORES = 8
NCH = 7              # x-buckets per core per dir
CH = 128             # query slots per chunk (partition dim)
NQP = NCH * CH       # 896
NPB = N // NCORES    # 810 queries per core per dir
NL = 4
HL = [60, 30, 15, 7]
WL = [108, 54, 27, 13]

INV_SQRT_D = np.float32(1.0) / np.sqrt(np.float32(D))


def plan_cfg(flow0, flow1, embt):
    """Host plan: 2D buckets, per-(chunk,dir,level) padded tile shapes."""
    ev = np.float32(np.asarray(embt).reshape(-1)[0])
    tsc = [np.float32(1.0) / ev, np.float32(1.0) / (np.float32(1.0) - ev)]
    wq = (np.arange(N) % W).astype(np.float32)
    hq = (np.arange(N) // W).astype(np.float32)

    xs, ys = [], []                     # [d] -> f32 coords per query
    buckets = [[[None] * NCH for _ in range(NCORES)] for _ in range(2)]
    r0 = np.zeros((NCORES, NCH, 2, NL), np.int64)
    c0 = np.zeros((NCORES, NCH, 2, NL), np.int64)
    rows = np.zeros((NCORES, NCH, 2, NL), np.int64)
    tws = np.zeros((NCORES, NCH, 2, NL), np.int64)
    for d in range(2):
        fl = (flow1 if d == 0 else flow0).reshape(2, N).astype(np.float32)
        x = (wq + fl[0] * tsc[d]).astype(np.float32)
        y = (hq + fl[1] * tsc[d]).astype(np.float32)
        xs.append(x)
        ys.append(y)
        order = np.argsort(y, kind="stable")
        for m in range(NCORES):
            band = order[m * NPB:(m + 1) * NPB]
            bx = band[np.argsort(x[band], kind="stable")]
            for c in range(NCH):
                qs = bx[NPB * c // NCH:NPB * (c + 1) // NCH]
                buckets[d][m][c] = qs
                for l in range(NL):
                    sc = np.float32(0.5 ** l)
                    fx = np.floor(x[qs] * sc)
                    fy = np.floor(y[qs] * sc)
                    # UNCLAMPED bounds: the tile always contains the full
                    # 8x8 window of every query (OOB rows/cols are host-
                    # packed zeros), so band reads never leave their block
                    a = int(fy.min()) - 3
                    b = int(fy.max()) + 4
                    e = int(fx.min()) - 3
                    f = int(fx.max()) + 4
                    r0[m, c, d, l] = a
                    c0[m, c, d, l] = e
                    rows[m, c, d, l] = b - a + 1
                    tws[m, c, d, l] = f - e + 1

    rows_p = rows.max(axis=0)           # [NCH, 2, NL] program shapes
    tws_p = tws.max(axis=0)

    loff = np.zeros((NCH, 2, NL + 1), np.int64)
    QS = np.zeros((NCH, 2), np.int64)
    for c in range(NCH):
        for d in range(2):
            off = 0
            for l in range(NL):
                loff[c, d, l] = off
                off += rows_p[c, d, l] * tws_p[c, d, l]
            loff[c, d, NL] = off
            QS[c, d] = (off + 31) // 32 * 32      # 4096-elem block alignment

    ext = CH * QS                                 # per-(c,d) DRAM tensor size
    # band geometry: one contiguous run of 7*tw+8 covers the 8x8 window
    BW = 7 * tws_p + 8                            # [NCH, 2, NL]
    BOFF = np.zeros((NCH, 2, NL), np.int64)       # band offsets in band tile
    BANDW = np.zeros(NCH, np.int64)
    for c in range(NCH):
        off = 0
        for d in range(2):
            for l in range(NL):
                BOFF[c, d, l] = off
                off += int(BW[c, d, l])
        BANDW[c] = off + 32          # slack for the 8*tw extraction views

    soff = np.zeros((NCH, 2), np.int64)           # imgslab offsets
    ST = [0, 0]
    for d in range(2):
        off = 0
        for c in range(NCH):
            soff[c, d] = off
            off += QS[c, d]
        ST[d] = off

    cfg = {
        "ev": ev, "tsc": tsc, "xs": xs, "ys": ys, "buckets": buckets,
        "r0": r0, "c0": c0, "rows_p": rows_p, "tws_p": tws_p,
        "loff": loff, "QS": QS, "ext": ext, "BW": BW, "BOFF": BOFF,
        "BANDW": BANDW, "soff": soff, "ST": ST,
    }
    cfg["key"] = (tuple(rows_p.ravel()), tuple(tws_p.ravel()),
                  tuple(QS.ravel()))
    return cfg


# static engine rotation tables (index by serial counters)
# drains: gpsimd cannot access PSUM -> DVE/Act only; Pool is reserved for
# the 56 band gathers (the bottleneck queue)
DRAIN_ENG = ["A", "A", "A", "V", "A", "A", "V", "A", "A", "V", "A", "A",
             "V", "A"]
WRITE_ENG = ["S", "A", "S", "A", "S", "A", "S", "A", "S", "A", "S", "A",
             "S", "A"]
IMG_ENG = ["S", "A", "S", "A", "S", "A", "S", "A", "S", "A", "S", "A",
           "S", "A"]


def build_nc(cfg):
    nc = bacc.Bacc()
    QS, ext, soff, ST = cfg["QS"], cfg["ext"], cfg["soff"], cfg["ST"]
    BW, BOFF, BANDW = cfg["BW"], cfg["BOFF"], cfg["BANDW"]
    tws_p = cfg["tws_p"]
    QSMAX = int(QS.max())
    BANDMAX = int(BANDW.max())

    f0qp = nc.declare_dram_parameter("f0q", [D, NQP], BF16, isOutput=False)
    f1qp = nc.declare_dram_parameter("f1q", [D, NQP], BF16, isOutput=False)
    img0p = nc.declare_dram_parameter("imgs0", [D, int(ST[0])], BF16,
                                      isOutput=False)
    img1p = nc.declare_dram_parameter("imgs1", [D, int(ST[1])], BF16,
                                      isOutput=False)
    offp = nc.declare_dram_parameter("offs", [128, NCH * 8], I32,
                                     isOutput=False)
    w0xp = nc.declare_dram_parameter("w0x", [128, NCH * 56], BF16,
                                     isOutput=False)
    w1xp = nc.declare_dram_parameter("w1x", [128, NCH * 56], BF16,
                                     isOutput=False)
    w0yp = nc.declare_dram_parameter("w0y", [128, NCH * 392], BF16,
                                     isOutput=False)
    w1yp = nc.declare_dram_parameter("w1y", [128, NCH * 392], BF16,
                                     isOutput=False)
    outp = nc.declare_dram_parameter("out", [NQP, 392], BF16, isOutput=True)

    with TileContext(nc) as tc:
        cpool = tc.alloc_tile_pool(name="cpool", bufs=1)
        dpool = tc.alloc_tile_pool(name="dpool", bufs=1, space="DRAM")
        ppool = tc.alloc_tile_pool(name="ppool", bufs=4, space="PSUM")
        spool = tc.alloc_tile_pool(name="spool", bufs=3)
        gpool = tc.alloc_tile_pool(name="gpool", bufs=3)
        bpool = tc.alloc_tile_pool(name="bpool", bufs=2)

        ENG = {"V": nc.vector, "A": nc.scalar, "P": nc.gpsimd, "S": nc.sync}

        # ---- input loads ----
        f0qs = cpool.tile([D, NQP], BF16, name="f0qs")
        f1qs = cpool.tile([D, NQP], BF16, name="f1qs")
        img0s = cpool.tile([D, int(ST[0])], BF16, name="img0s")
        img1s = cpool.tile([D, int(ST[1])], BF16, name="img1s")
        offs = cpool.tile([128, NCH * 8], I32, name="offs")
        w0xs = cpool.tile([128, NCH * 56], BF16, name="w0xs")
        w1xs = cpool.tile([128, NCH * 56], BF16, name="w1xs")
        w0ys = cpool.tile([128, NCH * 392], BF16, name="w0ys")
        w1ys = cpool.tile([128, NCH * 392], BF16, name="w1ys")

        def load_img(c, d):
            imgs, imgp = ((img0s, img0p), (img1s, img1p))[d]
            s0 = int(soff[c, d])
            q = int(QS[c, d])
            eng = nc.sync if d == 0 else nc.scalar
            eng.dma_start(out=imgs[:, s0:s0 + q], in_=imgp[:, s0:s0 + q])

        # head: only what chunk 0/1 need, so the first write lands early
        nc.scalar.dma_start(out=offs[:], in_=offp[:])
        load_img(0, 0)
        nc.sync.dma_start(out=f0qs[:], in_=f0qp[:])
        nc.scalar.dma_start(out=f1qs[:], in_=f1qp[:])
        load_img(0, 1)
        load_img(1, 0)
        load_img(1, 1)
        # warm the Act activation table after the head loads (it occupies
        # the Act queue for 1283ns; the first Act drain is at ~4us)
        zt = cpool.tile([128, 8], BF16, name="zt")
        zt2 = cpool.tile([128, 8], BF16, name="zt2")
        nc.vector.memset(zt[:], 0.0)
        nc.scalar.activation(out=zt2[:], in_=zt[:], func=ACT_COPY)

        # ---- per-(chunk,dir) DRAM tensors (offset-0 for indirect DMA) ----
        drams = {(c, d): dpool.tile([int(ext[c, d]), 1], BF16,
                                    name=f"pyr{c}_{d}")
                 for c in range(NCH) for d in range(2)}

        bands = {}

        def emit_compute(c):
            bandt = bpool.tile([128, BANDMAX], BF16, name="band", tag="band",
                               bufs=3)
            bands[c] = bandt
            if c + 2 < NCH:
                load_img(c + 2, 0)
                load_img(c + 2, 1)
            for d in range(2):
                q = int(QS[c, d])
                s0 = int(soff[c, d])
                lhsT = (f0qs if d == 0 else f1qs)[:, c * CH:(c + 1) * CH]
                imgs = img0s if d == 0 else img1s
                ps = ppool.tile([128, 1024], F32, name="ps", tag="ps")
                for p0 in range(0, q, 512):
                    plen = min(512, q - p0)
                    nc.tensor.matmul(
                        out=ps[:, p0:p0 + plen], lhsT=lhsT,
                        rhs=imgs[:, s0 + p0:s0 + p0 + plen],
                        start=True, stop=True)
                slab = spool.tile([128, QSMAX], BF16, name="slab", tag="slab")
                dr = drams[(c, d)]
                # drain d0 on DVE, d1 on Act; write d0 on SP, d1 on Act
                if d == 0:
                    nc.vector.tensor_copy(out=slab[:, 0:q], in_=ps[:, 0:q])
                    weng = nc.sync
                else:
                    nc.scalar.activation(out=slab[:, 0:q], in_=ps[:, 0:q],
                                         func=ACT_COPY)
                    weng = nc.scalar
                dst = bass.AP(dr[:, 0].tensor, dr[:, 0].offset,
                              [[q, CH], [1, q]])
                weng.dma_start(out=dst, in_=slab[:, 0:q])
                # per-level band gathers: ONE offset per partition (the only
                # indirect-DMA form the real DGE supports)
                inv = bass.AP(dr[:, 0].tensor, dr[:, 0].offset,
                              [[4096, int(ext[c, d]) // 4096], [1, 4096]])
                for l in range(NL):
                    g = d * 4 + l
                    bo = int(BOFF[c, d, l])
                    nc.gpsimd.indirect_dma_start(
                        out=bandt[:, bo:bo + int(BW[c, d, l])],
                        out_offset=None, in_=inv,
                        in_offset=IndirectOffsetOnAxis(
                            ap=offs[:, c * 8 + g:c * 8 + g + 1], axis=1))
            if c == 0:
                # weight tables: needed first by blend(0) (~9.5k)
                nc.sync.dma_start(out=w0xs[:], in_=w0xp[:])
                nc.scalar.dma_start(out=w1xs[:], in_=w1xp[:])
                nc.sync.dma_start(out=w0ys[:], in_=w0yp[:])
                nc.scalar.dma_start(out=w1ys[:], in_=w1yp[:])

        def emit_blend(c, split=False):
            bandt = bands.pop(c)
            pt = gpool.tile([128, 512], BF16, name="pt", tag="pt")
            pbv = pt[:].rearrange("p (g a b) -> p g a b", g=8, a=8)
            w0xv = w0xs[:, c * 56:(c + 1) * 56].rearrange(
                "p (g b) -> p g b", g=8).unsqueeze(2).broadcast_to(
                [128, 8, 8, 7])
            w1xv = w1xs[:, c * 56:(c + 1) * 56].rearrange(
                "p (g b) -> p g b", g=8).unsqueeze(2).broadcast_to(
                [128, 8, 8, 7])
            w0yv = w0ys[:, c * 392:(c + 1) * 392].rearrange(
                "p (g a b) -> p g a b", g=8, a=7)
            w1yv = w1ys[:, c * 392:(c + 1) * 392].rearrange(
                "p (g a b) -> p g a b", g=8, a=7)
            t0 = bpool.tile([128, 448], BF16, name="t0", tag="t0")
            t1 = bpool.tile([128, 448], BF16, name="t1", tag="t1")
            px = bpool.tile([128, 448], BF16, name="px", tag="px")
            t0v = t0[:].rearrange("p (g a b) -> p g a b", g=8, a=8)
            t1v = t1[:].rearrange("p (g a b) -> p g a b", g=8, a=8)
            pxv = px[:].rearrange("p (g a b) -> p g a b", g=8, a=8)
            u0 = bpool.tile([128, 392], BF16, name="u0", tag="u0")
            u1 = bpool.tile([128, 392], BF16, name="u1", tag="u1")
            ot = bpool.tile([128, 392], BF16, name="ot", tag="ot")
            u0v = u0[:].rearrange("p (g a b) -> p g a b", g=8, a=7)
            u1v = u1[:].rearrange("p (g a b) -> p g a b", g=8, a=7)
            otv = ot[:].rearrange("p (g a b) -> p g a b", g=8, a=7)

            def half(d):
                # extract this dir's 4 patches, then blend its g-slice
                for l in range(NL):
                    g = d * 4 + l
                    tw = int(tws_p[c, d, l])
                    bo = int(BOFF[c, d, l])
                    bv = bandt[:, bo:bo + 8 * tw].rearrange(
                        "p (a w) -> p a w", a=8)[:, :, 0:8]
                    nc.vector.tensor_copy(out=pbv[:, g], in_=bv)
                gs = slice(d * 4, d * 4 + 4)
                nc.vector.tensor_tensor(out=t0v[:, gs], in0=pbv[:, gs, :, 0:7],
                                        in1=w0xv[:, gs], op=OP.mult)
                nc.vector.tensor_tensor(out=t1v[:, gs], in0=pbv[:, gs, :, 1:8],
                                        in1=w1xv[:, gs], op=OP.mult)
                nc.vector.tensor_tensor(out=pxv[:, gs], in0=t0v[:, gs],
                                        in1=t1v[:, gs], op=OP.add)
                nc.vector.tensor_tensor(out=u0v[:, gs],
                                        in0=pxv[:, gs, 0:7, :],
                                        in1=w0yv[:, gs], op=OP.mult)
                nc.vector.tensor_tensor(out=u1v[:, gs],
                                        in0=pxv[:, gs, 1:8, :],
                                        in1=w1yv[:, gs], op=OP.mult)
                nc.vector.tensor_tensor(out=otv[:, gs], in0=u0v[:, gs],
                                        in1=u1v[:, gs], op=OP.add)
                nc.sync.dma_start(
                    out=outp[c * CH:(c + 1) * CH, d * 196:(d + 1) * 196],
                    in_=ot[:, d * 196:(d + 1) * 196])

            if split:
                half(0)
                half(1)
                return
            for d in range(2):
                for l in range(NL):
                    g = d * 4 + l
                    tw = int(tws_p[c, d, l])
                    bo = int(BOFF[c, d, l])
                    bv = bandt[:, bo:bo + 8 * tw].rearrange(
                        "p (a w) -> p a w", a=8)[:, :, 0:8]
                    nc.vector.tensor_copy(out=pbv[:, g], in_=bv)
            nc.vector.tensor_tensor(out=t0v, in0=pbv[:, :, :, 0:7],
                                    in1=w0xv, op=OP.mult)
            nc.vector.tensor_tensor(out=t1v, in0=pbv[:, :, :, 1:8],
                                    in1=w1xv, op=OP.mult)
            nc.vector.tensor_tensor(out=pxv, in0=t0v, in1=t1v, op=OP.add)
            nc.vector.tensor_tensor(out=u0v, in0=pxv[:, :, 0:7, :],
                                    in1=w0yv, op=OP.mult)
            nc.vector.tensor_tensor(out=u1v, in0=pxv[:, :, 1:8, :],
                                    in1=w1yv, op=OP.mult)
            nc.vector.tensor_tensor(out=otv, in0=u0v, in1=u1v, op=OP.add)
            nc.sync.dma_start(out=outp[c * CH:(c + 1) * CH, :], in_=ot[:])

        for c in range(NCH):
            emit_compute(c)
            if c >= 1:
                emit_blend(c - 1)
        emit_blend(NCH - 1, split=True)

        for pool in (bpool, gpool, spool, ppool, dpool, cpool):
            pool.release()

    nc.finalize()
    return nc


def host_tables(cfg, m):
    """Per-core gather offsets and folded bilinear/mask weights."""
    import ml_dtypes
    bf = ml_dtypes.bfloat16
    offs = np.zeros((128, NCH * 8), np.int32)
    w0x = np.zeros((128, NCH * 56), np.float32)
    w1x = np.zeros((128, NCH * 56), np.float32)
    w0y = np.zeros((128, NCH * 392), np.float32)
    w1y = np.zeros((128, NCH * 392), np.float32)
    QS, ext, loff, BW = cfg["QS"], cfg["ext"], cfg["loff"], cfg["BW"]
    av = np.arange(8)
    for c in range(NCH):
        for d in range(2):
            qs = cfg["buckets"][d][m][c]
            n = len(qs)
            rows_i = np.arange(n)
            q = int(QS[c, d])
            for l in range(NL):
                g = d * 4 + l
                sc = np.float32(0.5 ** l)
                xl = (cfg["xs"][d][qs] * sc).astype(np.float32)
                yl = (cfg["ys"][d][qs] * sc).astype(np.float32)
                fx = np.floor(xl)
                fy = np.floor(yl)
                wx = (xl - fx).astype(np.float32)
                wy = (yl - fy).astype(np.float32)
                stx = (fx - 3).astype(np.int64)
                sty = (fy - 3).astype(np.int64)
                r0 = int(cfg["r0"][m, c, d, l])
                cc0 = int(cfg["c0"][m, c, d, l])
                tw = int(cfg["tws_p"][c, d, l])
                # ONE band offset per query: 7*tw+8 contiguous elems cover
                # the 8 window rows at stride tw.  With unclamped tile
                # bounds the band is always inside this query's level
                # block; the clip below is a pure safety net.
                off = (rows_i * q + int(loff[c, d, l])
                       + (sty - r0) * tw + (stx - cc0))
                off = np.clip(off, 0, int(ext[c, d]) - int(BW[c, d, l]))
                offs[:n, c * 8 + g] = off
                # validity masks
                rv = ((sty[:, None] + av[None, :] >= 0)
                      & (sty[:, None] + av[None, :] <= HL[l] - 1)).astype(
                    np.float32)                                   # [n, 8]
                cv = ((stx[:, None] + av[None, :] >= 0)
                      & (stx[:, None] + av[None, :] <= WL[l] - 1)).astype(
                    np.float32)                                   # [n, 8]
                # x weights with col mask folded: [n, 7]
                w0x[:n, c * 56 + g * 7:c * 56 + g * 7 + 7] = \
                    (1.0 - wx)[:, None] * cv[:, 0:7]
                w1x[:n, c * 56 + g * 7:c * 56 + g * 7 + 7] = \
                    wx[:, None] * cv[:, 1:8]
                # y weights with row mask folded, materialized over b': [n,7,7]
                o = c * 392 + g * 49
                w0y[:n, o:o + 49] = np.repeat(
                    (1.0 - wy)[:, None] * rv[:, 0:7], 7, axis=1)
                w1y[:n, o:o + 49] = np.repeat(
                    wy[:, None] * rv[:, 1:8], 7, axis=1)
    return (offs, w0x.astype(bf), w1x.astype(bf), w0y.astype(bf),
            w1y.astype(bf))


def host_prepare(cfg, fmap0, fmap1, flow0, flow1, embt):
    import ml_dtypes
    bf = ml_dtypes.bfloat16
    f0 = fmap0.reshape(D, N).astype(np.float32)
    f1 = fmap1.reshape(D, N).astype(np.float32)

    def pyramid(f):
        cur = f.reshape(D, H, W)
        out = [cur]
        for l in range(1, NL):
            h, w = cur.shape[1] // 2, cur.shape[2] // 2
            cur = cur[:, :2 * h, :2 * w].reshape(D, h, 2, w, 2).mean((2, 4))
            out.append(cur)
        return out

    pyr = [pyramid(f1), pyramid(f0)]     # d=0 looks into fmap1, d=1 fmap0

    f0s = (f0 * INV_SQRT_D).astype(np.float32)
    f1s = (f1 * INV_SQRT_D).astype(np.float32)

    QS, soff, ST = cfg["QS"], cfg["soff"], cfg["ST"]
    loff = cfg["loff"]

    in_maps = []
    for m in range(NCORES):
        f0qA = np.zeros((D, NQP), np.float32)
        f1qA = np.zeros((D, NQP), np.float32)
        imgs = [np.zeros((D, int(ST[0])), np.float32),
                np.zeros((D, int(ST[1])), np.float32)]
        for c in range(NCH):
            for d in range(2):
                qs = cfg["buckets"][d][m][c]
                n = len(qs)
                if d == 0:
                    f0qA[:, c * CH:c * CH + n] = f0s[:, qs]
                else:
                    f1qA[:, c * CH:c * CH + n] = f1s[:, qs]
                for l in range(NL):
                    r0 = int(cfg["r0"][m, c, d, l])
                    cc0 = int(cfg["c0"][m, c, d, l])
                    rp = int(cfg["rows_p"][c, d, l])
                    tp = int(cfg["tws_p"][c, d, l])
                    src = pyr[d][l]
                    rs, re = max(0, r0), min(HL[l], r0 + rp)
                    cs, ce = max(0, cc0), min(WL[l], cc0 + tp)
                    blk = np.zeros((D, rp, tp), np.float32)
                    if re > rs and ce > cs:
                        blk[:, rs - r0:re - r0, cs - cc0:ce - cc0] = \
                            src[:, rs:re, cs:ce]
                    o = int(soff[c, d]) + int(loff[c, d, l])
                    imgs[d][:, o:o + rp * tp] = blk.reshape(D, rp * tp)
        offs, w0x, w1x, w0y, w1y = host_tables(cfg, m)
        in_maps.append({
            "f0q": np.ascontiguousarray(f0qA.astype(bf)),
            "f1q": np.ascontiguousarray(f1qA.astype(bf)),
            "imgs0": np.ascontiguousarray(imgs[0].astype(bf)),
            "imgs1": np.ascontiguousarray(imgs[1].astype(bf)),
            "offs": offs, "w0x": w0x, "w1x": w1x, "w0y": w0y, "w1y": w1y,
        })
    return in_maps


def assemble(cfg, results, flow0, flow1):
    corr = np.zeros((392, N), np.float32)
    res = [np.asarray(r["out"]).astype(np.float32) for r in results]
    for d in range(2):
        for m in range(NCORES):
            for c in range(NCH):
                qs = cfg["buckets"][d][m][c]
                n = len(qs)
                t = res[m][c * CH:c * CH + n].reshape(n, 8, 49)
                for l in range(NL):
                    ch0 = d * 196 + l * 49
                    corr[ch0:ch0 + 49, qs] = t[:, d * 4 + l, :].T
    full = np.concatenate(
        [corr.reshape(1, 392, H, W),
         flow0.astype(np.float32), flow1.astype(np.float32)], axis=1)
    return full


_CACHED = {}
LAST_NC = None


def kernel(fmap0, fmap1, flow0, flow1, embt):
    global LAST_NC
    from concourse.bass_utils import run_bass_kernel_spmd

    fmap0 = np.asarray(fmap0)
    fmap1 = np.asarray(fmap1)
    flow0 = np.asarray(flow0)
    flow1 = np.asarray(flow1)
    embt = np.asarray(embt)

    cfg = plan_cfg(flow0, flow1, embt)
    if cfg["key"] not in _CACHED:
        _CACHED[cfg["key"]] = build_nc(cfg)
    nc = _CACHED[cfg["key"]]
    LAST_NC = nc

    in_maps = host_prepare(cfg, fmap0, fmap1, flow0, flow1, embt)
    res = run_bass_kernel_spmd(nc, in_maps, core_ids=list(range(NCORES)))
    return assemble(cfg, res.results, flow0, flow1)


# revision 36
# speedup vs baseline: 1.0673x; 1.0099x over previous
"""Trainium2 Bass kernel: bidirectional RAFT-style correlation pyramid lookup
(AMT BidirCorrBlock + _corr_scale_lookup; B=1, D=128, H=60, W=108, r=3, L=4).

Design (8 NeuronCores, SPMD, no collectives):
  - Host computes lookup coords from flow/embt and 2D-buckets the queries:
    sort by y into 8 bands (one per core), then by x into 7 buckets of ~116
    queries.  Each bucket's correlation support is a small 2D tile per level
    (~600-820 cols total vs ~2700 for 1D y-slabs).
  - Host packs, per core, the needed image-tile columns of the (pooled)
    feature pyramids into flat "imgslab" tensors, so the device matmuls are
    plain contiguous [128q x QS] slabs (query features pre-scaled 1/sqrt(D)).
  - Per (chunk, dir): 2 matmuls -> one 2-bank PSUM tile -> ONE f32->bf16
    drain -> ONE bf16 DMA write to a per-(chunk,dir) DRAM tensor
    (query-major rows).
  - Per (chunk, dir, level): ONE indirect DMA band gather with a single
    offset per partition (the only form the real DGE supports): 7*tw+8
    contiguous elems cover a query's full 8x8 window at row stride tw.
    Because tile bounds are UNCLAMPED (OOB rows/cols are host-packed
    zeros), every band lies inside its own query's level block -- no DRAM
    zeroing or offset clamping needed.
  - Blend: 8 cheap DVE patch-extract copies + 6 DVE ops/chunk (separable
    bilinear); the OOB masks are folded into host-computed per-tap weights
    (w0x/w1x per (g,b'), w0y/w1y materialized per (g,a',b') so every op
    runs in 2x 16-bit mode).  The last chunk blends per-dir halves to
    shorten the tail.
  - Host reassembles [1, 396, 60, 108] (channel reorder + per-bucket query
    permutation + flow passthrough).  Program is compiled per bucket-shape
    key (cached across calls).
"""

import sys

import numpy as np

sys.path.insert(0, "/opt/trn_rl_repo")

import concourse.bass as bass
import concourse.bacc as bacc
import concourse.mybir as mybir
from concourse.bass import IndirectOffsetOnAxis
from concourse.tile import TileContext

F32 = mybir.dt.float32
BF16 = mybir.dt.bfloat16
I32 = mybir.dt.int32
OP = mybir.AluOpType
ACT_COPY = mybir.ActivationFunctionType.Copy

# problem geometry
H, W = 60, 108
N = H * W            # 6480
D = 128
NCORES = 8
NCH = 7              # x-buckets per core per dir
CH = 128             # query slots per chunk (partition dim)
NQP = NCH * CH       # 896
NPB = N // NCORES    # 810 queries per core per dir
NL = 4
HL = [60, 30, 15, 7]
WL = [108, 54, 27, 13]

INV_SQRT_D = np.float32(1.0) / np.sqrt(np.float32(D))


def plan_cfg(flow0, flow1, embt):
    """Host plan: 2D buckets, per-(chunk,dir,level) padded tile shapes."""
    ev = np.float32(np.asarray(embt).reshape(-1)[0])
    tsc = [np.float32(1.0) / ev, np.float32(1.0) / (np.float32(1.0) - ev)]
    wq = (np.arange(N) % W).astype(np.float32)
    hq = (np.arange(N) // W).astype(np.float32)

    xs, ys = [], []                     # [d] -> f32 coords per query
    buckets = [[[None] * NCH for _ in range(NCORES)] for _ in range(2)]
    r0 = np.zeros((NCORES, NCH, 2, NL), np.int64)
    c0 = np.zeros((NCORES, NCH, 2, NL), np.int64)
    rows = np.zeros((NCORES, NCH, 2, NL), np.int64)
    tws = np.zeros((NCORES, NCH, 2, NL), np.int64)
    for d in range(2):
        fl = (flow1 if d == 0 else flow0).reshape(2, N).astype(np.float32)
        x = (wq + fl[0] * tsc[d]).astype(np.float32)
        y = (hq + fl[1] * tsc[d]).astype(np.float32)
        xs.append(x)
        ys.append(y)
        order = np.argsort(y, kind="stable")
        for m in range(NCORES):
            band = order[m * NPB:(m + 1) * NPB]
            bx = band[np.argsort(x[band], kind="stable")]
            for c in range(NCH):
                qs = bx[NPB * c // NCH:NPB * (c + 1) // NCH]
                buckets[d][m][c] = qs
                for l in range(NL):
                    sc = np.float32(0.5 ** l)
                    fx = np.floor(x[qs] * sc)
                    fy = np.floor(y[qs] * sc)
                    # UNCLAMPED bounds: the tile always contains the full
                    # 8x8 window of every query (OOB rows/cols are host-
                    # packed zeros), so band reads never leave their block
                    a = int(fy.min()) - 3
                    b = int(fy.max()) + 4
                    e = int(fx.min()) - 3
                    f = int(fx.max()) + 4
                    r0[m, c, d, l] = a
                    c0[m, c, d, l] = e
                    rows[m, c, d, l] = b - a + 1
                    tws[m, c, d, l] = f - e + 1

    rows_p = rows.max(axis=0)           # [NCH, 2, NL] program shapes
    tws_p = tws.max(axis=0)

    loff = np.zeros((NCH, 2, NL + 1), np.int64)
    QS = np.zeros((NCH, 2), np.int64)
    for c in range(NCH):
        for d in range(2):
            off = 0
            for l in range(NL):
                loff[c, d, l] = off
                off += rows_p[c, d, l] * tws_p[c, d, l]
            loff[c, d, NL] = off
            QS[c, d] = (off + 31) // 32 * 32      # 4096-elem block alignment

    ext = CH * QS                                 # per-(c,d) DRAM tensor size
    # band geometry: one contiguous run of 7*tw+8 covers the 8x8 window
    BW = 7 * tws_p + 8                            # [NCH, 2, NL]
    BOFF = np.zeros((NCH, 2, NL), np.int64)       # band offsets in band tile
    BANDW = np.zeros(NCH, np.int64)
    for c in range(NCH):
        off = 0
        for d in range(2):
            for l in range(NL):
                BOFF[c, d, l] = off
                off += int(BW[c, d, l])
        BANDW[c] = off + 32          # slack for the 8*tw extraction views

    soff = np.zeros((NCH, 2), np.int64)           # imgslab offsets
    ST = [0, 0]
    for d in range(2):
        off = 0
        for c in range(NCH):
            soff[c, d] = off
            off += QS[c, d]
        ST[d] = off

    cfg = {
        "ev": ev, "tsc": tsc, "xs": xs, "ys": ys, "buckets": buckets,
        "r0": r0, "c0": c0, "rows_p": rows_p, "tws_p": tws_p,
        "loff": loff, "QS": QS, "ext": ext, "BW": BW, "BOFF": BOFF,
        "BANDW": BANDW, "soff": soff, "ST": ST,
    }
    cfg["key"] = (tuple(rows_p.ravel()), tuple(tws_p.ravel()),
                  tuple(QS.ravel()))
    return cfg


# static engine rotation tables (index by serial counters)
# drains: gpsimd cannot access PSUM -> DVE/Act only; Pool is reserved for
# the 56 band gathers (the bottleneck queue)
DRAIN_ENG = ["A", "A", "A", "V", "A", "A", "V", "A", "A", "V", "A", "A",
             "V", "A"]
WRITE_ENG = ["S", "A", "S", "A", "S", "A", "S", "A", "S", "A", "S", "A",
             "S", "A"]
IMG_ENG = ["S", "A", "S", "A", "S", "A", "S", "A", "S", "A", "S", "A",
           "S", "A"]


def build_nc(cfg):
    nc = bacc.Bacc()
    QS, ext, soff, ST = cfg["QS"], cfg["ext"], cfg["soff"], cfg["ST"]
    BW, BOFF, BANDW = cfg["BW"], cfg["BOFF"], cfg["BANDW"]
    tws_p = cfg["tws_p"]
    QSMAX = int(QS.max())
    BANDMAX = int(BANDW.max())

    f0qp = nc.declare_dram_parameter("f0q", [D, NQP], BF16, isOutput=False)
    f1qp = nc.declare_dram_parameter("f1q", [D, NQP], BF16, isOutput=False)
    img0p = nc.declare_dram_parameter("imgs0", [D, int(ST[0])], BF16,
                                      isOutput=False)
    img1p = nc.declare_dram_parameter("imgs1", [D, int(ST[1])], BF16,
                                      isOutput=False)
    offp = nc.declare_dram_parameter("offs", [128, NCH * 8], I32,
                                     isOutput=False)
    w0xp = nc.declare_dram_parameter("w0x", [128, NCH * 56], BF16,
                                     isOutput=False)
    w1xp = nc.declare_dram_parameter("w1x", [128, NCH * 56], BF16,
                                     isOutput=False)
    w0yp = nc.declare_dram_parameter("w0y", [128, NCH * 392], BF16,
                                     isOutput=False)
    w1yp = nc.declare_dram_parameter("w1y", [128, NCH * 392], BF16,
                                     isOutput=False)
    outp = nc.declare_dram_parameter("out", [NQP, 392], BF16, isOutput=True)

    with TileContext(nc) as tc:
        cpool = tc.alloc_tile_pool(name="cpool", bufs=1)
        dpool = tc.alloc_tile_pool(name="dpool", bufs=1, space="DRAM")
        ppool = tc.alloc_tile_pool(name="ppool", bufs=4, space="PSUM")
        spool = tc.alloc_tile_pool(name="spool", bufs=3)
        gpool = tc.alloc_tile_pool(name="gpool", bufs=3)
        bpool = tc.alloc_tile_pool(name="bpool", bufs=2)

        ENG = {"V": nc.vector, "A": nc.scalar, "P": nc.gpsimd, "S": nc.sync}

        # ---- input loads ----
        f0qs = cpool.tile([D, NQP], BF16, name="f0qs")
        f1qs = cpool.tile([D, NQP], BF16, name="f1qs")
        img0s = cpool.tile([D, int(ST[0])], BF16, name="img0s")
        img1s = cpool.tile([D, int(ST[1])], BF16, name="img1s")
        offs = cpool.tile([128, NCH * 8], I32, name="offs")
        w0xs = cpool.tile([128, NCH * 56], BF16, name="w0xs")
        w1xs = cpool.tile([128, NCH * 56], BF16, name="w1xs")
        w0ys = cpool.tile([128, NCH * 392], BF16, name="w0ys")
        w1ys = cpool.tile([128, NCH * 392], BF16, name="w1ys")

        def load_img(c, d):
            imgs, imgp = ((img0s, img0p), (img1s, img1p))[d]
            s0 = int(soff[c, d])
            q = int(QS[c, d])
            eng = nc.sync if d == 0 else nc.scalar
            eng.dma_start(out=imgs[:, s0:s0 + q], in_=imgp[:, s0:s0 + q])

        # process chunks smallest-first: the pipeline primes on the
        # cheapest img->mm->drain->write chain
        order = sorted(range(NCH), key=lambda c: int(QS[c, 0] + QS[c, 1]))

        # head: only what the first two chunks need, so the first write
        # lands early
        nc.scalar.dma_start(out=offs[:], in_=offp[:])
        load_img(order[0], 0)
        nc.sync.dma_start(out=f0qs[:], in_=f0qp[:])
        nc.scalar.dma_start(out=f1qs[:], in_=f1qp[:])
        load_img(order[0], 1)
        load_img(order[1], 0)
        load_img(order[1], 1)
        # warm the Act activation table after the head loads (it occupies
        # the Act queue for 1283ns; the first Act drain is at ~4us)
        zt = cpool.tile([128, 8], BF16, name="zt")
        zt2 = cpool.tile([128, 8], BF16, name="zt2")
        nc.vector.memset(zt[:], 0.0)
        nc.scalar.activation(out=zt2[:], in_=zt[:], func=ACT_COPY)

        # ---- per-(chunk,dir) DRAM tensors (offset-0 for indirect DMA) ----
        drams = {(c, d): dpool.tile([int(ext[c, d]), 1], BF16,
                                    name=f"pyr{c}_{d}")
                 for c in range(NCH) for d in range(2)}

        bands = {}

        def emit_compute(c, prefetch):
            bandt = bpool.tile([128, BANDMAX], BF16, name="band", tag="band",
                               bufs=3)
            bands[c] = bandt
            if prefetch is not None:
                load_img(prefetch, 0)
                load_img(prefetch, 1)
            for d in range(2):
                q = int(QS[c, d])
                s0 = int(soff[c, d])
                lhsT = (f0qs if d == 0 else f1qs)[:, c * CH:(c + 1) * CH]
                imgs = img0s if d == 0 else img1s
                ps = ppool.tile([128, 1024], F32, name="ps", tag="ps")
                for p0 in range(0, q, 512):
                    plen = min(512, q - p0)
                    nc.tensor.matmul(
                        out=ps[:, p0:p0 + plen], lhsT=lhsT,
                        rhs=imgs[:, s0 + p0:s0 + p0 + plen],
                        start=True, stop=True)
                slab = spool.tile([128, QSMAX], BF16, name="slab", tag="slab")
                dr = drams[(c, d)]
                # drain d0 on DVE, d1 on Act; write d0 on SP, d1 on Act
                if d == 0:
                    nc.vector.tensor_copy(out=slab[:, 0:q], in_=ps[:, 0:q])
                    weng = nc.sync
                else:
                    nc.scalar.activation(out=slab[:, 0:q], in_=ps[:, 0:q],
                                         func=ACT_COPY)
                    weng = nc.scalar
                dst = bass.AP(dr[:, 0].tensor, dr[:, 0].offset,
                              [[q, CH], [1, q]])
                weng.dma_start(out=dst, in_=slab[:, 0:q])
                # per-level band gathers: ONE offset per partition (the only
                # indirect-DMA form the real DGE supports)
                inv = bass.AP(dr[:, 0].tensor, dr[:, 0].offset,
                              [[4096, int(ext[c, d]) // 4096], [1, 4096]])
                for l in range(NL):
                    g = d * 4 + l
                    bo = int(BOFF[c, d, l])
                    nc.gpsimd.indirect_dma_start(
                        out=bandt[:, bo:bo + int(BW[c, d, l])],
                        out_offset=None, in_=inv,
                        in_offset=IndirectOffsetOnAxis(
                            ap=offs[:, c * 8 + g:c * 8 + g + 1], axis=1))
            if c == 0:
                # weight tables: needed first by blend(0) (~9.5k)
                nc.sync.dma_start(out=w0xs[:], in_=w0xp[:])
                nc.scalar.dma_start(out=w1xs[:], in_=w1xp[:])
                nc.sync.dma_start(out=w0ys[:], in_=w0yp[:])
                nc.scalar.dma_start(out=w1ys[:], in_=w1yp[:])

        def emit_blend(c, split=False):
            bandt = bands.pop(c)
            pt = gpool.tile([128, 512], BF16, name="pt", tag="pt")
            pbv = pt[:].rearrange("p (g a b) -> p g a b", g=8, a=8)
            w0xv = w0xs[:, c * 56:(c + 1) * 56].rearrange(
                "p (g b) -> p g b", g=8).unsqueeze(2).broadcast_to(
                [128, 8, 8, 7])
            w1xv = w1xs[:, c * 56:(c + 1) * 56].rearrange(
                "p (g b) -> p g b", g=8).unsqueeze(2).broadcast_to(
                [128, 8, 8, 7])
            w0yv = w0ys[:, c * 392:(c + 1) * 392].rearrange(
                "p (g a b) -> p g a b", g=8, a=7)
            w1yv = w1ys[:, c * 392:(c + 1) * 392].rearrange(
                "p (g a b) -> p g a b", g=8, a=7)
            t0 = bpool.tile([128, 448], BF16, name="t0", tag="t0")
            t1 = bpool.tile([128, 448], BF16, name="t1", tag="t1")
            px = bpool.tile([128, 448], BF16, name="px", tag="px")
            t0v = t0[:].rearrange("p (g a b) -> p g a b", g=8, a=8)
            t1v = t1[:].rearrange("p (g a b) -> p g a b", g=8, a=8)
            pxv = px[:].rearrange("p (g a b) -> p g a b", g=8, a=8)
            u0 = bpool.tile([128, 392], BF16, name="u0", tag="u0")
            u1 = bpool.tile([128, 392], BF16, name="u1", tag="u1")
            ot = bpool.tile([128, 392], BF16, name="ot", tag="ot")
            u0v = u0[:].rearrange("p (g a b) -> p g a b", g=8, a=7)
            u1v = u1[:].rearrange("p (g a b) -> p g a b", g=8, a=7)
            otv = ot[:].rearrange("p (g a b) -> p g a b", g=8, a=7)

            def half(d):
                # extract this dir's 4 patches, then blend its g-slice
                for l in range(NL):
                    g = d * 4 + l
                    tw = int(tws_p[c, d, l])
                    bo = int(BOFF[c, d, l])
                    bv = bandt[:, bo:bo + 8 * tw].rearrange(
                        "p (a w) -> p a w", a=8)[:, :, 0:8]
                    nc.vector.tensor_copy(out=pbv[:, g], in_=bv)
                gs = slice(d * 4, d * 4 + 4)
                nc.vector.tensor_tensor(out=t0v[:, gs], in0=pbv[:, gs, :, 0:7],
                                        in1=w0xv[:, gs], op=OP.mult)
                nc.vector.tensor_tensor(out=t1v[:, gs], in0=pbv[:, gs, :, 1:8],
                                        in1=w1xv[:, gs], op=OP.mult)
                nc.vector.tensor_tensor(out=pxv[:, gs], in0=t0v[:, gs],
                                        in1=t1v[:, gs], op=OP.add)
                nc.vector.tensor_tensor(out=u0v[:, gs],
                                        in0=pxv[:, gs, 0:7, :],
                                        in1=w0yv[:, gs], op=OP.mult)
                nc.vector.tensor_tensor(out=u1v[:, gs],
                                        in0=pxv[:, gs, 1:8, :],
                                        in1=w1yv[:, gs], op=OP.mult)
                nc.vector.tensor_tensor(out=otv[:, gs], in0=u0v[:, gs],
                                        in1=u1v[:, gs], op=OP.add)
                nc.sync.dma_start(
                    out=outp[c * CH:(c + 1) * CH, d * 196:(d + 1) * 196],
                    in_=ot[:, d * 196:(d + 1) * 196])

            if split:
                half(0)
                half(1)
                return
            for d in range(2):
                for l in range(NL):
                    g = d * 4 + l
                    tw = int(tws_p[c, d, l])
                    bo = int(BOFF[c, d, l])
                    bv = bandt[:, bo:bo + 8 * tw].rearrange(
                        "p (a w) -> p a w", a=8)[:, :, 0:8]
                    nc.vector.tensor_copy(out=pbv[:, g], in_=bv)
            nc.vector.tensor_tensor(out=t0v, in0=pbv[:, :, :, 0:7],
                                    in1=w0xv, op=OP.mult)
            nc.vector.tensor_tensor(out=t1v, in0=pbv[:, :, :, 1:8],
                                    in1=w1xv, op=OP.mult)
            nc.vector.tensor_tensor(out=pxv, in0=t0v, in1=t1v, op=OP.add)
            nc.vector.tensor_tensor(out=u0v, in0=pxv[:, :, 0:7, :],
                                    in1=w0yv, op=OP.mult)
            nc.vector.tensor_tensor(out=u1v, in0=pxv[:, :, 1:8, :],
                                    in1=w1yv, op=OP.mult)
            nc.vector.tensor_tensor(out=otv, in0=u0v, in1=u1v, op=OP.add)
            nc.sync.dma_start(out=outp[c * CH:(c + 1) * CH, :], in_=ot[:])

        for i, c in enumerate(order):
            pf = order[i + 2] if i + 2 < NCH else None
            emit_compute(c, pf)
            if i >= 1:
                emit_blend(order[i - 1])
        emit_blend(order[NCH - 1], split=True)

        for pool in (bpool, gpool, spool, ppool, dpool, cpool):
            pool.release()

    nc.finalize()
    return nc


def host_tables(cfg, m):
    """Per-core gather offsets and folded bilinear/mask weights."""
    import ml_dtypes
    bf = ml_dtypes.bfloat16
    offs = np.zeros((128, NCH * 8), np.int32)
    w0x = np.zeros((128, NCH * 56), np.float32)
    w1x = np.zeros((128, NCH * 56), np.float32)
    w0y = np.zeros((128, NCH * 392), np.float32)
    w1y = np.zeros((128, NCH * 392), np.float32)
    QS, ext, loff, BW = cfg["QS"], cfg["ext"], cfg["loff"], cfg["BW"]
    av = np.arange(8)
    for c in range(NCH):
        for d in range(2):
            qs = cfg["buckets"][d][m][c]
            n = len(qs)
            rows_i = np.arange(n)
            q = int(QS[c, d])
            for l in range(NL):
                g = d * 4 + l
                sc = np.float32(0.5 ** l)
                xl = (cfg["xs"][d][qs] * sc).astype(np.float32)
                yl = (cfg["ys"][d][qs] * sc).astype(np.float32)
                fx = np.floor(xl)
                fy = np.floor(yl)
                wx = (xl - fx).astype(np.float32)
                wy = (yl - fy).astype(np.float32)
                stx = (fx - 3).astype(np.int64)
                sty = (fy - 3).astype(np.int64)
                r0 = int(cfg["r0"][m, c, d, l])
                cc0 = int(cfg["c0"][m, c, d, l])
                tw = int(cfg["tws_p"][c, d, l])
                # ONE band offset per query: 7*tw+8 contiguous elems cover
                # the 8 window rows at stride tw.  With unclamped tile
                # bounds the band is always inside this query's level
                # block; the clip below is a pure safety net.
                off = (rows_i * q + int(loff[c, d, l])
                       + (sty - r0) * tw + (stx - cc0))
                off = np.clip(off, 0, int(ext[c, d]) - int(BW[c, d, l]))
                offs[:n, c * 8 + g] = off
                # validity masks
                rv = ((sty[:, None] + av[None, :] >= 0)
                      & (sty[:, None] + av[None, :] <= HL[l] - 1)).astype(
                    np.float32)                                   # [n, 8]
                cv = ((stx[:, None] + av[None, :] >= 0)
                      & (stx[:, None] + av[None, :] <= WL[l] - 1)).astype(
                    np.float32)                                   # [n, 8]
                # x weights with col mask folded: [n, 7]
                w0x[:n, c * 56 + g * 7:c * 56 + g * 7 + 7] = \
                    (1.0 - wx)[:, None] * cv[:, 0:7]
                w1x[:n, c * 56 + g * 7:c * 56 + g * 7 + 7] = \
                    wx[:, None] * cv[:, 1:8]
                # y weights with row mask folded, materialized over b': [n,7,7]
                o = c * 392 + g * 49
                w0y[:n, o:o + 49] = np.repeat(
                    (1.0 - wy)[:, None] * rv[:, 0:7], 7, axis=1)
                w1y[:n, o:o + 49] = np.repeat(
                    wy[:, None] * rv[:, 1:8], 7, axis=1)
    return (offs, w0x.astype(bf), w1x.astype(bf), w0y.astype(bf),
            w1y.astype(bf))


def host_prepare(cfg, fmap0, fmap1, flow0, flow1, embt):
    import ml_dtypes
    bf = ml_dtypes.bfloat16
    f0 = fmap0.reshape(D, N).astype(np.float32)
    f1 = fmap1.reshape(D, N).astype(np.float32)

    def pyramid(f):
        cur = f.reshape(D, H, W)
        out = [cur]
        for l in range(1, NL):
            h, w = cur.shape[1] // 2, cur.shape[2] // 2
            cur = cur[:, :2 * h, :2 * w].reshape(D, h, 2, w, 2).mean((2, 4))
            out.append(cur)
        return out

    pyr = [pyramid(f1), pyramid(f0)]     # d=0 looks into fmap1, d=1 fmap0

    f0s = (f0 * INV_SQRT_D).astype(np.float32)
    f1s = (f1 * INV_SQRT_D).astype(np.float32)

    QS, soff, ST = cfg["QS"], cfg["soff"], cfg["ST"]
    loff = cfg["loff"]

    in_maps = []
    for m in range(NCORES):
        f0qA = np.zeros((D, NQP), np.float32)
        f1qA = np.zeros((D, NQP), np.float32)
        imgs = [np.zeros((D, int(ST[0])), np.float32),
                np.zeros((D, int(ST[1])), np.float32)]
        for c in range(NCH):
            for d in range(2):
                qs = cfg["buckets"][d][m][c]
                n = len(qs)
                if d == 0:
                    f0qA[:, c * CH:c * CH + n] = f0s[:, qs]
                else:
                    f1qA[:, c * CH:c * CH + n] = f1s[:, qs]
                for l in range(NL):
                    r0 = int(cfg["r0"][m, c, d, l])
                    cc0 = int(cfg["c0"][m, c, d, l])
                    rp = int(cfg["rows_p"][c, d, l])
                    tp = int(cfg["tws_p"][c, d, l])
                    src = pyr[d][l]
                    rs, re = max(0, r0), min(HL[l], r0 + rp)
                    cs, ce = max(0, cc0), min(WL[l], cc0 + tp)
                    blk = np.zeros((D, rp, tp), np.float32)
                    if re > rs and ce > cs:
                        blk[:, rs - r0:re - r0, cs - cc0:ce - cc0] = \
                            src[:, rs:re, cs:ce]
                    o = int(soff[c, d]) + int(loff[c, d, l])
                    imgs[d][:, o:o + rp * tp] = blk.reshape(D, rp * tp)
        offs, w0x, w1x, w0y, w1y = host_tables(cfg, m)
        in_maps.append({
            "f0q": np.ascontiguousarray(f0qA.astype(bf)),
            "f1q": np.ascontiguousarray(f1qA.astype(bf)),
            "imgs0": np.ascontiguousarray(imgs[0].astype(bf)),
            "imgs1": np.ascontiguousarray(imgs[1].astype(bf)),
            "offs": offs, "w0x": w0x, "w1x": w1x, "w0y": w0y, "w1y": w1y,
        })
    return in_maps


def assemble(cfg, results, flow0, flow1):
    corr = np.zeros((392, N), np.float32)
    res = [np.asarray(r["out"]).astype(np.float32) for r in results]
    for d in range(2):
        for m in range(NCORES):
            for c in range(NCH):
                qs = cfg["buckets"][d][m][c]
                n = len(qs)
                t = res[m][c * CH:c * CH + n].reshape(n, 8, 49)
                for l in range(NL):
                    ch0 = d * 196 + l * 49
                    corr[ch0:ch0 + 49, qs] = t[:, d * 4 + l, :].T
    full = np.concatenate(
        [corr.reshape(1, 392, H, W),
         flow0.astype(np.float32), flow1.astype(np.float32)], axis=1)
    return full


_CACHED = {}
LAST_NC = None


def kernel(fmap0, fmap1, flow0, flow1, embt):
    global LAST_NC
    from concourse.bass_utils import run_bass_kernel_spmd

    fmap0 = np.asarray(fmap0)
    fmap1 = np.asarray(fmap1)
    flow0 = np.asarray(flow0)
    flow1 = np.asarray(flow1)
    embt = np.asarray(embt)

    cfg = plan_cfg(flow0, flow1, embt)
    if cfg["key"] not in _CACHED:
        _CACHED[cfg["key"]] = build_nc(cfg)
    nc = _CACHED[cfg["key"]]
    LAST_NC = nc

    in_maps = host_prepare(cfg, fmap0, fmap1, flow0, flow1, embt)
    res = run_bass_kernel_spmd(nc, in_maps, core_ids=list(range(NCORES)))
    return assemble(cfg, res.results, flow0, flow1)
